# revision 3
# baseline (speedup 1.0000x reference)
"""Trainium2 Bass kernel for nn_BoneLinear: out = x @ W^T + pooled(x) @ disha.

Identity used: pooled(x) @ disha == x @ A where A[j, o] = disha[j % 64, o]
(vertical tiling of disha).  So the whole module is one dense matmul:
    out = x @ (W^T + tile(disha, 16))

Per-core pipeline (all 8 cores run this over their own batch shard):
  1. Setup: load W naturally, PE-transpose it (4 transposes packed per PSUM
     bank as one accumulation group), add the partition-tiled disha, and round
     to fp16 -> W_eff^T resident in SBUF [128, 8, 1024].
  2. Steady state, software-pipelined over 16 token-tile PAIRS: HWDGE-load x
     in 1 MB pairs -> cast f32->fp16 -> ONE xbar DMA-transpose per pair
     (SBUF->SBUF, 2-byte dtype, out AP [128, 16, 128]: out[p, mid, f] =
     in[f, mid*128+p], i.e. exactly chunked k-on-partitions layout) ->
     16 accumulating fp16 matmuls per tile (N=512) -> ACT/DVE copies
     PSUM->SBUF -> one batched store per pair.
     The PE runs ONLY matmuls (transposes moved to the DMA xbar, which rides
     otherwise-idle SBUF-fabric capacity), so PE ~= 2.1us/tile while DMA
     paces at the HBM roofline ~2.8us/tile.

Sharding: pure data-parallel over batch (B=8 -> one batch element per core).
Each core reads its x shard [4096, 1024], full weight and disha, and writes
its output shard [4096, 1024].  No collectives.
"""

import sys
import os

for _p in ("/opt/trn_rl_repo", "/root/.axon_site/_ro/trn_rl_repo"):
    if os.path.isdir(_p) and _p not in sys.path:
        sys.path.insert(0, _p)

import numpy as np

import concourse.bass as bass
import concourse.mybir as mybir
import concourse.tile as tile
from concourse import bacc
from concourse.bass_utils import run_bass_kernel_spmd
from concourse.masks import make_identity

# Problem shapes (hardcoded per contract)
B, S, D_IN, D_OUT, R = 8, 4096, 1024, 1024, 64
N_CORES = 8
P = 128
KO = D_IN // P          # 8 contraction chunks of 128
OC = D_OUT // P         # 8 output chunks of 128 (for W transpose)
MT = S // P             # 32 token tiles per core
NF = 512                # matmul moving free dim (one PSUM bank of fp32)
NT = D_OUT // NF        # 2 n-tiles

F32 = mybir.dt.float32
F32R = mybir.dt.float32r
F16 = mybir.dt.float16
MM_DT = F16


def build_bass(reps: int = 1, loop: int = 1, pw: int = 2,
               acc_bufs: int = 4, gp_cast: bool = False,
               tp_engine: str = "scalar"):
    """reps>1 (python-unrolled) or loop>1 (hardware For_i) repeat the
    steady-state compute inside the NEFF; used only for wall-clock
    differencing in benchmarks (the graded kernel uses reps=1, loop=1)."""
    nc = bacc.Bacc("TRN2", target_bir_lowering=False, debug=False, num_devices=1)
    x_ap = nc.dram_tensor("x", [S, D_IN], F32, kind="ExternalInput").ap()
    w_ap = nc.dram_tensor("w", [D_OUT, D_IN], F32, kind="ExternalInput").ap()
    d_ap = nc.dram_tensor("disha", [R, D_OUT], F32, kind="ExternalInput").ap()
    out_ap = nc.dram_tensor("out", [S, D_OUT], F32, kind="ExternalOutput").ap()

    PW = pw                 # token tiles per staging pair
    NP = MT // PW           # number of pairs

    with tile.TileContext(nc) as tc:
        with (
            tc.tile_pool(name="const", bufs=1) as const,
            tc.tile_pool(name="wp", bufs=1) as wpool,
            tc.tile_pool(name="xp", bufs=4) as xpool,
            tc.tile_pool(name="xh", bufs=3) as xhpool,
            tc.tile_pool(name="xtp", bufs=3) as xtpool,
            tc.tile_pool(name="op", bufs=3) as opool,
        ):
            ident = const.tile([P, P], MM_DT)
            make_identity(nc, ident)

            # disha tiled twice on partitions: disha2[p, :] = disha[p % 64, :]
            disha2f = const.tile([P, D_OUT], F32)
            nc.sync.dma_start(disha2f[0:R, :], d_ap[:, :])
            nc.sync.dma_start(disha2f[R : 2 * R, :], d_ap[:, :])
            disha2 = const.tile([P, D_OUT], MM_DT)
            nc.any.tensor_copy(disha2[:], disha2f[:])

            # Build W_eff^T[p + 128*kc, oc*128 + q] = W[q(within oc), p(of kc)] + disha2[p]
            # 4 PE transposes packed per PSUM bank (one accumulation group),
            # then a single wide DVE add per bank.
            GRP = NF // P  # 4 transposes per bank
            w_eff = wpool.tile([P, KO, D_OUT], MM_DT)
            with (
                tc.tile_pool(name="wnat", bufs=1) as wnat_pool,
                tc.tile_pool(name="pstp", bufs=4, space="PSUM") as psum_tp,
            ):
                w_nat = wnat_pool.tile([P, OC, D_IN], F32)
                w_nath = wnat_pool.tile([P, OC, D_IN], MM_DT)
                w_src = w_ap.rearrange("(oc p) d -> p oc d", p=P)
                for kc in range(KO):
                    nc.sync.dma_start(
                        w_nat[:, :, kc * P : (kc + 1) * P],
                        w_src[:, :, kc * P : (kc + 1) * P],
                    )
                    nc.any.tensor_copy(
                        w_nath[:, :, kc * P : (kc + 1) * P],
                        w_nat[:, :, kc * P : (kc + 1) * P],
                    )
                for kc in range(KO):
                    for og in range(OC // GRP):
                        pst = psum_tp.tile([P, NF], MM_DT, tag="tp")
                        for j in range(GRP):
                            oc = og * GRP + j
                            nc.tensor.matmul(
                                pst[:, j * P : (j + 1) * P],
                                w_nath[:, oc, kc * P : (kc + 1) * P],
                                ident[:],
                                is_transpose=True,
                                start=(j == 0),
                                stop=(j == GRP - 1),
                            )
                        nc.vector.tensor_add(
                            w_eff[:, kc, og * NF : (og + 1) * NF],
                            pst[:],
                            disha2[:, og * NF : (og + 1) * NF],
                        )

            tp_dma = nc.scalar if tp_engine == "scalar" else nc.sync

            # Main loop over token-tile pairs
            import contextlib

            with tc.tile_pool(name="psacc", bufs=acc_bufs, space="PSUM") as psum_acc:
                loop_cm = (
                    tc.For_i(0, loop, 1) if loop > 1 else contextlib.nullcontext()
                )
                with loop_cm:
                    for rep in range(reps):

                        def emit_load(mp, rep=rep):
                            """DMA PW token tiles at once; cast to fp16."""
                            src = x_ap[
                                mp * PW * P : (mp + 1) * PW * P, :
                            ].rearrange("(two p) d -> p two d", two=PW)
                            x_h = xhpool.tile(
                                [P, PW, D_IN], MM_DT, tag="x_h",
                                name=f"xh_{rep}_{mp}",
                            )
                            if gp_cast:
                                nc.gpsimd.dma_start(x_h[:], src)
                            else:
                                x_t = xpool.tile(
                                    [P, PW, D_IN], F32, tag="x_t",
                                    name=f"x_{rep}_{mp}",
                                )
                                nc.sync.dma_start(x_t[:], src)
                                nc.any.tensor_copy(x_h[:], x_t[:])
                            return x_h

                        def emit_tp(x_h, mp, rep=rep):
                            """One xbar DMA-transpose for the whole pair:
                            xT[p, t*KO+kc, m] = x_h[m, t, kc*128+p]."""
                            xT = xtpool.tile(
                                [P, PW * KO, P], MM_DT, tag="xT",
                                name=f"xT_{rep}_{mp}",
                            )
                            tp_dma.dma_start(xT[:], x_h[:], transpose=True)
                            return xT

                        xh = {0: emit_load(0)}
                        if NP > 1:
                            xh[1] = emit_load(1)
                        xT = {0: emit_tp(xh[0], 0)}
                        for mp in range(NP):
                            if mp + 2 < NP:
                                xh[mp + 2] = emit_load(mp + 2)
                            if mp + 1 < NP:
                                xT[mp + 1] = emit_tp(xh[mp + 1], mp + 1)
                            o_sb = opool.tile(
                                [P, PW, D_OUT], F32, tag="o2",
                                name=f"o_{rep}_{mp}",
                            )
                            for t in range(PW):
                                m = mp * PW + t
                                pss = [
                                    psum_acc.tile(
                                        [P, NF], F32, tag=f"acc{n}",
                                        name=f"acc_{rep}_{m}_{n}",
                                    )
                                    for n in range(NT)
                                ]
                                for kc in range(KO):
                                    for n in range(NT):
                                        nc.tensor.matmul(
                                            pss[n][:],
                                            xT[mp][:, t * KO + kc, :],
                                            w_eff[:, kc, n * NF : (n + 1) * NF],
                                            start=(kc == 0),
                                            stop=(kc == KO - 1),
                                        )
                                for n in range(NT):
                                    nc.any.tensor_copy(
                                        o_sb[:, t, n * NF : (n + 1) * NF],
                                        pss[n][:],
                                    )
                            nc.sync.dma_start(
                                out_ap[
                                    mp * PW * P : (mp + 1) * PW * P, :
                                ].rearrange("(two p) d -> p two d", two=PW),
                                o_sb[:],
                            )
                            xh.pop(mp, None)
                            xT.pop(mp, None)

    nc.compile()
    return nc


def kernel(x: np.ndarray, weight: np.ndarray, disha: np.ndarray) -> np.ndarray:
    assert x.shape == (B, S, D_IN) and weight.shape == (D_OUT, D_IN)
    assert disha.shape == (R, D_OUT)
    x = np.ascontiguousarray(x, dtype=np.float32)
    weight = np.ascontiguousarray(weight, dtype=np.float32)
    disha = np.ascontiguousarray(disha, dtype=np.float32)
    in_maps = [
        {"x": x[c], "w": weight, "disha": disha} for c in range(N_CORES)
    ]
    # The axon-proxied exec occasionally dies with NRT_EXEC_UNIT_UNRECOVERABLE
    # on an otherwise-good NEFF; retry a couple of times with a fresh build.
    last_exc = None
    for attempt in range(3):
        try:
            nc = build_bass()
            res = run_bass_kernel_spmd(
                nc, in_maps, core_ids=list(range(N_CORES))
            )
            break
        except Exception as e:  # noqa: BLE001
            last_exc = e
            import time as _time

            _time.sleep(5.0 * (attempt + 1))
    else:
        raise last_exc
    out = np.stack([res.results[c]["out"] for c in range(N_CORES)], axis=0)
    return out


if __name__ == "__main__":
    rng = np.random.default_rng(0)
    x = rng.standard_normal((B, S, D_IN), dtype=np.float32)
    w = (rng.standard_normal((D_OUT, D_IN), dtype=np.float32) / 32.0).astype(
        np.float32
    )
    d = (rng.standard_normal((R, D_OUT), dtype=np.float32) * 0.01).astype(np.float32)
    out = kernel(x=x, weight=w, disha=d)
    print(out.shape, out.dtype)


# revision 17
# speedup vs baseline: 1.1327x; 1.1327x over previous
"""Trainium2 Bass kernel for nn_BoneLinear: out = x @ W^T + pooled(x) @ disha.

Identity used: pooled(x) @ disha == x @ A where A[j, o] = disha[j % 64, o]
(vertical tiling of disha).  So the whole module is one dense matmul:
    out = x @ (W^T + tile(disha, 16))

Per-core pipeline (all 8 cores run this over their own batch shard):
  1. Setup: load W naturally, PE-transpose it (4 transposes packed per PSUM
     bank as one accumulation group), add the partition-tiled disha, and round
     to fp16 -> W_eff^T resident in SBUF [128, 8, 1024].
  2. Steady state, software-pipelined one tile deep over 32 token tiles:
     HWDGE-load x in 1 MB pairs on the SP ring -> cast f32->fp16 (DVE/ACT)
     -> PE-transpose the NEXT tile's 8 [128,128] chunks in groups of 4
     packed per fp16 PSUM bank (one accumulation group each; ~100ns per
     transpose when pipelined) -> one DVE copy per group to SBUF ->
     16 accumulating fp16 matmuls for the CURRENT tile (N=512, kc-outer so
     each LDWEIGHTS covers 2 matmuls and pulls ahead via the PE reorder
     window) -> ACT/DVE copies PSUM->SBUF -> batched pair store on the
     ACT HWDGE ring (loads and stores on separate rings overlap; measured
     split-ring HBM floor ~100us/pass vs ~121us single-ring).
     PE is the bottleneck at ~4.3us/tile (matmul roofline 8192 cyc/tile
     = 3.4us at 2.4 GHz + ~0.7us transposes); DMA (~3.1us/tile) hides.
     DMA-xbar transposes were tried and rejected: sharing the 16 SDMA
     engines with the HBM streams collapses HBM burst efficiency
     (stream+xbar measured 209us/pass vs 100+29 separately).

Sharding: pure data-parallel over batch (B=8 -> one batch element per core).
Each core reads its x shard [4096, 1024], full weight and disha, and writes
its output shard [4096, 1024].  No collectives.  fp16 operands measured
relmax ~3.3e-4 vs the fp32 reference.
"""

import sys
import os

for _p in ("/opt/trn_rl_repo", "/root/.axon_site/_ro/trn_rl_repo"):
    if os.path.isdir(_p) and _p not in sys.path:
        sys.path.insert(0, _p)

import numpy as np

import concourse.bass as bass
import concourse.mybir as mybir
import concourse.tile as tile
from concourse import bacc
from concourse.bass_utils import run_bass_kernel_spmd
from concourse.masks import make_identity

# Problem shapes (hardcoded per contract)
B, S, D_IN, D_OUT, R = 8, 4096, 1024, 1024, 64
N_CORES = 8
P = 128
KO = D_IN // P          # 8 contraction chunks of 128
OC = D_OUT // P         # 8 output chunks of 128 (for W transpose)
MT = S // P             # 32 token tiles per core
NF = 512                # matmul moving free dim (one PSUM bank of fp32)
NT = D_OUT // NF        # 2 n-tiles

F32 = mybir.dt.float32
F32R = mybir.dt.float32r
F16 = mybir.dt.float16
MM_DT = F16


def build_bass(reps: int = 1, loop: int = 1, pw: int = 2,
               acc_bufs: int = 3, tp_bufs: int = 2, tp_grp: int = 4,
               xbar_frac: int = 0, st_engine: str = "scalar",
               deep: bool = False, fused_acc: bool = False,
               interleave_tp: bool = False):
    """reps>1 (python-unrolled) or loop>1 (hardware For_i) repeat the
    steady-state compute inside the NEFF; used only for wall-clock
    differencing in benchmarks (the graded kernel uses reps=1, loop=1)."""
    nc = bacc.Bacc("TRN2", target_bir_lowering=False, debug=False, num_devices=1)
    x_ap = nc.dram_tensor("x", [S, D_IN], F32, kind="ExternalInput").ap()
    w_ap = nc.dram_tensor("w", [D_OUT, D_IN], F32, kind="ExternalInput").ap()
    d_ap = nc.dram_tensor("disha", [R, D_OUT], F32, kind="ExternalInput").ap()
    out_ap = nc.dram_tensor("out", [S, D_OUT], F32, kind="ExternalOutput").ap()

    PW = pw                 # token tiles per staging pair
    NP = MT // PW           # number of pairs

    with tile.TileContext(nc) as tc:
        with (
            tc.tile_pool(name="const", bufs=1) as const,
            tc.tile_pool(name="wp", bufs=1) as wpool,
            tc.tile_pool(name="xp", bufs=5 if deep else 4) as xpool,
            tc.tile_pool(name="xh", bufs=4 if deep else 3) as xhpool,
            tc.tile_pool(name="xtp", bufs=4 if deep else 3) as xtpool,
            tc.tile_pool(name="op", bufs=4 if deep else 3) as opool,
        ):
            ident = const.tile([P, P], MM_DT)
            make_identity(nc, ident)

            # disha tiled twice on partitions: disha2[p, :] = disha[p % 64, :]
            disha2f = const.tile([P, D_OUT], F32)
            nc.sync.dma_start(disha2f[0:R, :], d_ap[:, :])
            nc.sync.dma_start(disha2f[R : 2 * R, :], d_ap[:, :])
            disha2 = const.tile([P, D_OUT], MM_DT)
            nc.any.tensor_copy(disha2[:], disha2f[:])

            # Build W_eff^T[p + 128*kc, oc*128 + q] = W[q(within oc), p(of kc)] + disha2[p]
            # 4 PE transposes packed per PSUM bank (one accumulation group),
            # then a single wide DVE add per bank.
            GRP = NF // P  # 4 transposes per bank
            w_eff = wpool.tile([P, KO, D_OUT], MM_DT)
            with (
                tc.tile_pool(name="wnat", bufs=1) as wnat_pool,
                tc.tile_pool(name="pstp", bufs=4, space="PSUM") as psum_tp,
            ):
                w_nat = wnat_pool.tile([P, OC, D_IN], F32)
                w_nath = wnat_pool.tile([P, OC, D_IN], MM_DT)
                w_src = w_ap.rearrange("(oc p) d -> p oc d", p=P)
                for kc in range(KO):
                    nc.sync.dma_start(
                        w_nat[:, :, kc * P : (kc + 1) * P],
                        w_src[:, :, kc * P : (kc + 1) * P],
                    )
                    nc.any.tensor_copy(
                        w_nath[:, :, kc * P : (kc + 1) * P],
                        w_nat[:, :, kc * P : (kc + 1) * P],
                    )
                for kc in range(KO):
                    for og in range(OC // GRP):
                        pst = psum_tp.tile([P, NF], MM_DT, tag="tp")
                        for j in range(GRP):
                            oc = og * GRP + j
                            nc.tensor.matmul(
                                pst[:, j * P : (j + 1) * P],
                                w_nath[:, oc, kc * P : (kc + 1) * P],
                                ident[:],
                                is_transpose=True,
                                start=(j == 0),
                                stop=(j == GRP - 1),
                            )
                        nc.vector.tensor_add(
                            w_eff[:, kc, og * NF : (og + 1) * NF],
                            pst[:],
                            disha2[:, og * NF : (og + 1) * NF],
                        )

            st_dma = nc.scalar if st_engine == "scalar" else nc.sync

            # Main loop over token-tile pairs
            import contextlib

            with (
                tc.tile_pool(name="pstp2", bufs=tp_bufs, space="PSUM") as psum_tp2,
                tc.tile_pool(name="psacc", bufs=acc_bufs, space="PSUM") as psum_acc,
            ):
                loop_cm = (
                    tc.For_i(0, loop, 1) if loop > 1 else contextlib.nullcontext()
                )
                with loop_cm:
                    for rep in range(reps):

                        def emit_load(mp, rep=rep):
                            """DMA PW token tiles at once; cast to fp16."""
                            src = x_ap[
                                mp * PW * P : (mp + 1) * PW * P, :
                            ].rearrange("(two p) d -> p two d", two=PW)
                            x_t = xpool.tile(
                                [P, PW, D_IN], F32, tag="x_t",
                                name=f"x_{rep}_{mp}",
                            )
                            nc.sync.dma_start(x_t[:], src)
                            x_h = xhpool.tile(
                                [P, PW, D_IN], MM_DT, tag="x_h",
                                name=f"xh_{rep}_{mp}",
                            )
                            nc.any.tensor_copy(x_h[:], x_t[:])
                            return x_h

                        def emit_tp(xh, m, rep=rep):
                            """Transpose token tile m on the PE: groups of
                            tp_grp 128x128 transposes packed into one fp16
                            PSUM bank, then one DVE copy per group.  Every
                            xbar_frac-th tile goes via the DMA xbar instead
                            (on the store ring)."""
                            x_hs = xh[m // PW][:, m % PW, :]
                            xT = xtpool.tile(
                                [P, KO, P], MM_DT, tag="xT",
                                name=f"xT_{rep}_{m}",
                            )
                            if xbar_frac and m % xbar_frac == 0:
                                st_dma.dma_start(xT[:], x_hs, transpose=True)
                                return xT
                            for g in range(KO // tp_grp):
                                pst = psum_tp2.tile(
                                    [P, tp_grp * P], MM_DT, tag="tp",
                                    name=f"tp_{rep}_{m}_{g}",
                                )
                                for j in range(tp_grp):
                                    kc = g * tp_grp + j
                                    nc.tensor.matmul(
                                        pst[:, j * P : (j + 1) * P],
                                        x_hs[:, kc * P : (kc + 1) * P],
                                        ident[:],
                                        is_transpose=True,
                                        start=(j == 0),
                                        stop=(j == tp_grp - 1),
                                    )
                                nc.vector.tensor_copy(
                                    xT[:, g * tp_grp : (g + 1) * tp_grp], pst[:]
                                )
                            return xT

                        def emit_mms_tp_interleaved(xh, m, xT_cur, o_sb, t,
                                                    rep=rep):
                            """MMs of tile m with the PE transposes of tile
                            m+1 interleaved chunk-wise, so every transpose
                            LDWEIGHTS hides behind an MM stream via the PE
                            reorder window."""
                            nxt = m + 1 < MT
                            if nxt:
                                x_hs = xh[(m + 1) // PW][:, (m + 1) % PW, :]
                                xT_next = xtpool.tile(
                                    [P, KO, P], MM_DT, tag="xT",
                                    name=f"xT_{rep}_{m + 1}",
                                )
                            else:
                                xT_next = None
                            pss = [
                                psum_acc.tile(
                                    [P, NF], F32, tag=f"acc{n}",
                                    name=f"acc_{rep}_{m}_{n}",
                                )
                                for n in range(NT)
                            ]
                            pst = None
                            for kc in range(KO):
                                if nxt:
                                    g, j = kc // tp_grp, kc % tp_grp
                                    if j == 0:
                                        pst = psum_tp2.tile(
                                            [P, tp_grp * P], MM_DT, tag="tp",
                                            name=f"tp_{rep}_{m + 1}_{g}",
                                        )
                                    nc.tensor.matmul(
                                        pst[:, j * P : (j + 1) * P],
                                        x_hs[:, kc * P : (kc + 1) * P],
                                        ident[:],
                                        is_transpose=True,
                                        start=(j == 0),
                                        stop=(j == tp_grp - 1),
                                    )
                                for n in range(NT):
                                    nc.tensor.matmul(
                                        pss[n][:],
                                        xT_cur[:, kc],
                                        w_eff[:, kc, n * NF : (n + 1) * NF],
                                        start=(kc == 0),
                                        stop=(kc == KO - 1),
                                    )
                                if nxt and kc % tp_grp == tp_grp - 1:
                                    g = kc // tp_grp
                                    nc.vector.tensor_copy(
                                        xT_next[:, g * tp_grp : (g + 1) * tp_grp],
                                        pst[:],
                                    )
                            for n in range(NT):
                                nc.any.tensor_copy(
                                    o_sb[:, t, n * NF : (n + 1) * NF],
                                    pss[n][:],
                                )
                            return xT_next

                        xh = {0: emit_load(0)}
                        if NP > 1:
                            xh[1] = emit_load(1)
                        xT_cur = emit_tp(xh, 0)
                        for mp in range(NP):
                            if mp + 2 < NP:
                                xh[mp + 2] = emit_load(mp + 2)
                            o_sb = opool.tile(
                                [P, PW, D_OUT], F32, tag="o2",
                                name=f"o_{rep}_{mp}",
                            )
                            for t in range(PW):
                                m = mp * PW + t
                                if interleave_tp:
                                    xT_next = emit_mms_tp_interleaved(
                                        xh, m, xT_cur, o_sb, t
                                    )
                                    xT_cur = xT_next
                                    continue
                                xT_next = (
                                    emit_tp(xh, m + 1) if m + 1 < MT else None
                                )
                                if fused_acc:
                                    acc2 = psum_acc.tile(
                                        [P, D_OUT], F32, tag="acc",
                                        name=f"acc_{rep}_{m}",
                                    )
                                    pss = [
                                        acc2[:, n * NF : (n + 1) * NF]
                                        for n in range(NT)
                                    ]
                                else:
                                    pss = [
                                        psum_acc.tile(
                                            [P, NF], F32, tag=f"acc{n}",
                                            name=f"acc_{rep}_{m}_{n}",
                                        )[:]
                                        for n in range(NT)
                                    ]
                                for kc in range(KO):
                                    for n in range(NT):
                                        nc.tensor.matmul(
                                            pss[n],
                                            xT_cur[:, kc],
                                            w_eff[:, kc, n * NF : (n + 1) * NF],
                                            start=(kc == 0),
                                            stop=(kc == KO - 1),
                                        )
                                if fused_acc:
                                    nc.any.tensor_copy(o_sb[:, t, :], acc2[:])
                                else:
                                    for n in range(NT):
                                        nc.any.tensor_copy(
                                            o_sb[:, t, n * NF : (n + 1) * NF],
                                            pss[n],
                                        )
                                xT_cur = xT_next
                            st_dma.dma_start(
                                out_ap[
                                    mp * PW * P : (mp + 1) * PW * P, :
                                ].rearrange("(two p) d -> p two d", two=PW),
                                o_sb[:],
                            )
                            xh.pop(mp, None)

    nc.compile()
    return nc


def kernel(x: np.ndarray, weight: np.ndarray, disha: np.ndarray) -> np.ndarray:
    assert x.shape == (B, S, D_IN) and weight.shape == (D_OUT, D_IN)
    assert disha.shape == (R, D_OUT)
    x = np.ascontiguousarray(x, dtype=np.float32)
    weight = np.ascontiguousarray(weight, dtype=np.float32)
    disha = np.ascontiguousarray(disha, dtype=np.float32)
    in_maps = [
        {"x": x[c], "w": weight, "disha": disha} for c in range(N_CORES)
    ]
    # The axon-proxied exec occasionally dies with NRT_EXEC_UNIT_UNRECOVERABLE
    # on an otherwise-good NEFF; retry a couple of times with a fresh build.
    last_exc = None
    for attempt in range(3):
        try:
            nc = build_bass()
            res = run_bass_kernel_spmd(
                nc, in_maps, core_ids=list(range(N_CORES))
            )
            break
        except Exception as e:  # noqa: BLE001
            last_exc = e
            import time as _time

            _time.sleep(5.0 * (attempt + 1))
    else:
        raise last_exc
    out = np.stack([res.results[c]["out"] for c in range(N_CORES)], axis=0)
    return out


if __name__ == "__main__":
    rng = np.random.default_rng(0)
    x = rng.standard_normal((B, S, D_IN), dtype=np.float32)
    w = (rng.standard_normal((D_OUT, D_IN), dtype=np.float32) / 32.0).astype(
        np.float32
    )
    d = (rng.standard_normal((R, D_OUT), dtype=np.float32) * 0.01).astype(np.float32)
    out = kernel(x=x, weight=w, disha=d)
    print(out.shape, out.dtype)
